# revision 14
# baseline (speedup 1.0000x reference)
"""Trainium2 Bass kernel for EnhancedMultiHeadAttention (B=2, S=2048, DM=1024, H=16).

Sharding: 8 NeuronCores = 2 batches x 4 query-row blocks of 512 rows. Each
core computes K/V for its whole batch (4x redundant; collectives on this
runtime cost ~120us per op and stall the PE long enough to halve the HAM
clock for the rest of the kernel — measured, so local recompute wins), plus
attention and epilogue for its own 512 query rows.

v4 over the 379us v1: the output and gate projections are folded into the
attention loop. The host precomputes Wgo = Wg @ Wo and bgo = Wg @ bo + bg;
as each head pair's context finishes, 8+8 matmuls accumulate its
contribution to out^T and gate_z^T in SBUF. This removes v1's serial
epilogue matmul chain (outT + orow + gate, ~75us of tail PE) entirely.
The epilogue is elementwise-only in transposed [DM, 512] layout (sigmoid,
gate mix, layernorm with cross-partition stats via a ones-matmul), and
y^T is transposed on the host.

Schedule (from v1): one fused window. Q^T projected up front; each head
pair's K rows projected just-in-time; V column halves projected into
SBUF-resident tiles, with pair 0's V tiles fused into its attention t-loop
so the ScalarE GELU stream starts as early as possible. Projection and
incremental-fold matmuls fill PE slack underneath the GELU stream, which
also keeps the PE HAM clock un-throttled.

Precision: fp32r matmuls except attn @ v in bf16 (PSUM column packing of
the head pair). softmax(attention_weights) folded into Wv/bv on the host.
"""
import math
import os
import sys

import numpy as np

for _p in ("/opt/trn_rl_repo", "/opt/pypackages"):
    if _p not in sys.path:
        sys.path.append(_p)

import concourse.bass as bass
import concourse.mybir as mybir
import concourse.tile as tile
from concourse import bacc
from concourse.bass_utils import run_bass_kernel_spmd

F32R = mybir.dt.float32r
F32 = mybir.dt.float32
BF16 = mybir.dt.bfloat16
AF = mybir.ActivationFunctionType
ALU = mybir.AluOpType

B, S, DM, H = 2, 2048, 1024, 16
HD = DM // H                  # 64
SQ = 512                      # query rows per core
NP = 128                      # partitions
KC = DM // NP                 # 8 contraction chunks
NT = S // NP                  # 16 key/value tiles
NPAIR = H // 2                # 8 head pairs
NST = SQ // NP                # 4 row tiles
N512 = 512
NQ = 512                      # v-projection column half width
SCALE = 1.0 / math.sqrt(HD)
EPS = 1e-5

_CACHE = {}
_TRACE = [False]
_LAST_RESULT = [None]


def _bcast(ap_1d, p=NP):
    return bass.AP(tensor=ap_1d.tensor, offset=ap_1d.offset,
                   ap=[[0, p]] + list(ap_1d.ap))


def _build():
    nc = bacc.Bacc("TRN2", target_bir_lowering=False, debug=False)

    xT_d = nc.dram_tensor("xT", [DM, S], F32R, kind="ExternalInput").ap()
    wkT_d = nc.dram_tensor("wkT", [DM, DM], F32R, kind="ExternalInput").ap()
    wvT_d = nc.dram_tensor("wvT", [DM, DM], F32R, kind="ExternalInput").ap()
    wqT_d = nc.dram_tensor("wqT", [DM, DM], F32R, kind="ExternalInput").ap()
    woT_d = nc.dram_tensor("woT", [DM, DM], F32R, kind="ExternalInput").ap()
    wgoT_d = nc.dram_tensor("wgoT", [DM, DM], F32R, kind="ExternalInput").ap()
    bq_d = nc.dram_tensor("bq", [DM], F32, kind="ExternalInput").ap()
    bk_d = nc.dram_tensor("bk", [DM], F32, kind="ExternalInput").ap()
    bv_d = nc.dram_tensor("bv", [DM], F32, kind="ExternalInput").ap()
    bo_d = nc.dram_tensor("bo", [DM], F32, kind="ExternalInput").ap()
    bgo_d = nc.dram_tensor("bgo", [DM], F32, kind="ExternalInput").ap()
    gam_d = nc.dram_tensor("gam", [DM], F32, kind="ExternalInput").ap()
    bet_d = nc.dram_tensor("bet", [DM], F32, kind="ExternalInput").ap()
    y_d = nc.dram_tensor("y", [DM, SQ], F32, kind="ExternalOutput").ap()

    xT_v = xT_d.rearrange("(c p) s -> p c s", p=NP)
    wk_v = wkT_d.rearrange("(c p) d -> p c d", p=NP)
    wv_v = wvT_d.rearrange("(c p) d -> p c d", p=NP)
    wq_v = wqT_d.rearrange("(c p) d -> p c d", p=NP)
    wo_v = woT_d.rearrange("(c p) d -> p c d", p=NP)
    wgo_v = wgoT_d.rearrange("(c p) d -> p c d", p=NP)
    y_v = y_d.rearrange("(c p) s -> p c s", p=NP)

    with tile.TileContext(nc) as tc:
        with tc.tile_pool(name="pers", bufs=1) as pers, \
             tc.tile_pool(name="acc", bufs=1) as acc:
            bq_sb = pers.tile([NP, KC], F32)
            bk_sb = pers.tile([NP, KC], F32)
            bo_sb = pers.tile([NP, KC], F32)
            bgo_sb = pers.tile([NP, KC], F32)
            gam_sb = pers.tile([NP, KC], F32)
            bet_sb = pers.tile([NP, KC], F32)
            nc.sync.dma_start(out=bq_sb, in_=bq_d.rearrange("(c p) -> p c", p=NP))
            nc.sync.dma_start(out=bk_sb, in_=bk_d.rearrange("(c p) -> p c", p=NP))
            nc.sync.dma_start(out=bo_sb, in_=bo_d.rearrange("(c p) -> p c", p=NP))
            nc.sync.dma_start(out=bgo_sb, in_=bgo_d.rearrange("(c p) -> p c", p=NP))
            nc.sync.dma_start(out=gam_sb, in_=gam_d.rearrange("(c p) -> p c", p=NP))
            nc.sync.dma_start(out=bet_sb, in_=bet_d.rearrange("(c p) -> p c", p=NP))
            bv_bc = pers.tile([NP, DM], F32)
            nc.sync.dma_start(out=bv_bc, in_=_bcast(bv_d))
            eps_sb = pers.tile([NP, 1], F32)
            nc.vector.memset(eps_sb, EPS)
            ones_f32 = pers.tile([NP, NP], F32)
            nc.vector.memset(ones_f32, 1.0)
            ones_sb = pers.tile([NP, NP], F32R)
            nc.gpsimd.dma_start(out=ones_sb, in_=ones_f32)

            outT_acc = acc.tile([NP, KC, SQ], F32R)   # out^T accumulator
            gateT_acc = acc.tile([NP, KC, SQ], F32R)  # gate logits^T accum

            with tc.tile_pool(name="xres", bufs=1) as xres, \
                 tc.tile_pool(name="qres", bufs=1) as qres, \
                 tc.tile_pool(name="wvp", bufs=1) as wvp, \
                 tc.tile_pool(name="wsl", bufs=2) as wsl, \
                 tc.tile_pool(name="wop", bufs=2) as wop, \
                 tc.tile_pool(name="kpp", bufs=2) as kpp, \
                 tc.tile_pool(name="vqp", bufs=2) as vqp, \
                 tc.tile_pool(name="ctxp", bufs=2) as ctxp, \
                 tc.tile_pool(name="attp", bufs=3) as attp, \
                 tc.tile_pool(name="pp", bufs=2, space="PSUM") as pp, \
                 tc.tile_pool(name="scop", bufs=2, space="PSUM") as scop, \
                 tc.tile_pool(name="cxp", bufs=2, space="PSUM") as cxp:
                xT_sb = xres.tile([NP, KC, S], F32R)
                qT_sb = qres.tile([NP, KC, SQ], BF16)

                v_q = [None] * 2
                wv_sbs = [None] * 2

                def v_open(q):
                    wv_sb = wvp.tile([NP, KC, NQ], F32R, tag="wv", name="wv_sb")
                    nc.gpsimd.dma_start(out=wv_sb,
                                        in_=wv_v[:, :, q * NQ:(q + 1) * NQ])
                    wv_sbs[q] = wv_sb
                    vq = vqp.tile([NP, NT, NQ], BF16, tag="vq", name="vq")
                    v_q[q] = vq

                def v_tt(q, tt):
                    ps_t = pp.tile([NP, NQ], F32, tag="pj", name="ps_t")
                    for kc in range(KC):
                        nc.tensor.matmul(
                            ps_t,
                            xT_sb[:, kc, tt * NP:(tt + 1) * NP],
                            wv_sbs[q][:, kc, :],
                            start=(kc == 0), stop=(kc == KC - 1))
                    nc.vector.tensor_add(
                        v_q[q][:, tt, :], ps_t, bv_bc[:, q * NQ:(q + 1) * NQ])

                kpairs = [None] * NPAIR
                wk_tiles = {}

                def wk_dma(p):
                    wk_sl = wsl.tile([NP, KC, NP], F32R, tag="wk", name="wk_sl")
                    nc.sync.dma_start(out=wk_sl,
                                      in_=wk_v[:, :, p * NP:(p + 1) * NP])
                    wk_tiles[p] = wk_sl

                def prepare(p):
                    # kpair[d, t] = sum_k Wk[d, k] x[t, k] + bk[d]
                    if p not in wk_tiles:
                        wk_dma(p)
                    wk_sl = wk_tiles[p]
                    kpair = kpp.tile([NP, S], BF16, tag="kp", name="kpair")
                    for ts in range(S // N512):
                        ps_t = pp.tile([NP, N512], F32, tag="pj", name="ps_t")
                        for kc in range(KC):
                            nc.tensor.matmul(
                                ps_t,
                                wk_sl[:, kc, :],
                                xT_sb[:, kc, ts * N512:(ts + 1) * N512],
                                start=(kc == 0), stop=(kc == KC - 1))
                        nc.vector.tensor_scalar_add(
                            kpair[:, ts * N512:(ts + 1) * N512], ps_t,
                            bk_sb[:, p:p + 1])
                    kpairs[p] = kpair

                def attn(p, pre_t=None):
                    kpair = kpairs[p]
                    vq = v_q[p // 4]
                    c0 = (p % 4) * NP
                    wo_sl = wop.tile([NP, DM], F32R, tag="wo", name="wo_sl")
                    nc.sync.dma_start(out=wo_sl, in_=wo_v[:, p, :])
                    wgo_sl = wop.tile([NP, DM], F32R, tag="wo", name="wgo_sl")
                    nc.sync.dma_start(out=wgo_sl, in_=wgo_v[:, p, :])
                    ctx_ps = cxp.tile([NP, SQ], F32, tag="cx", name="ctx_ps")
                    for t in range(NT):
                        if pre_t is not None:
                            pre_t(t)
                        sco = scop.tile([NP, 2 * SQ], F32, tag="sc", name="sco")
                        nc.tensor.matmul(sco[:, 0:SQ],
                                         kpair[0:64, t * NP:(t + 1) * NP],
                                         qT_sb[0:64, p, :],
                                         start=True, stop=True,
                                         tile_position=(0, 0))
                        nc.tensor.matmul(sco[:, SQ:2 * SQ],
                                         kpair[64:128, t * NP:(t + 1) * NP],
                                         qT_sb[64:128, p, :],
                                         start=True, stop=True,
                                         tile_position=(64, 0))
                        att_t = attp.tile([NP, 2 * SQ], BF16, tag="at",
                                          name="att_t")
                        nc.scalar.activation(out=att_t, in_=sco, func=AF.Gelu,
                                             scale=SCALE)
                        nc.tensor.matmul(ctx_ps[0:64, :], vq[:, t, c0:c0 + 64],
                                         att_t[:, 0:SQ],
                                         start=(t == 0), stop=(t == NT - 1),
                                         tile_position=(0, 0))
                        nc.tensor.matmul(ctx_ps[64:128, :],
                                         vq[:, t, c0 + 64:c0 + NP],
                                         att_t[:, SQ:2 * SQ],
                                         start=(t == 0), stop=(t == NT - 1),
                                         tile_position=(0, 64))
                    ctx_sb = ctxp.tile([NP, SQ], F32R, tag="cs", name="ctx_sb")
                    nc.vector.tensor_copy(ctx_sb, ctx_ps)
                    # fold pair p into out^T and gate^T
                    for dt in range(KC):
                        ps_o = pp.tile([NP, SQ], F32, tag="pj", name="ps_o")
                        nc.tensor.matmul(ps_o, wo_sl[:, dt * NP:(dt + 1) * NP],
                                         ctx_sb, start=True, stop=True)
                        if p == 0:
                            nc.vector.tensor_scalar_add(
                                outT_acc[:, dt, :], ps_o, bo_sb[:, dt:dt + 1])
                        else:
                            nc.vector.tensor_add(
                                outT_acc[:, dt, :], ps_o, outT_acc[:, dt, :])
                        ps_g = pp.tile([NP, SQ], F32, tag="pj", name="ps_g")
                        nc.tensor.matmul(ps_g, wgo_sl[:, dt * NP:(dt + 1) * NP],
                                         ctx_sb, start=True, stop=True)
                        if p == 0:
                            nc.vector.tensor_scalar_add(
                                gateT_acc[:, dt, :], ps_g, bgo_sb[:, dt:dt + 1])
                        else:
                            nc.vector.tensor_add(
                                gateT_acc[:, dt, :], ps_g, gateT_acc[:, dt, :])

                # opening DMA order: wk(0), xT query-cols chunk, then the
                # Q projection (wq slices land right behind), then rest of xT
                wk_dma(0)
                for kc in range(KC):
                    nc.sync.dma_start(
                        out=xT_sb[:, kc, 0:N512], in_=xT_v[:, kc, 0:N512])
                for dt in range(KC):
                    wq_sl = wsl.tile([NP, KC, NP], F32R, tag="wq", name="wq_sl")
                    nc.sync.dma_start(out=wq_sl,
                                      in_=wq_v[:, :, dt * NP:(dt + 1) * NP])
                    ps_q = pp.tile([NP, SQ], F32, tag="pj", name="ps_q")
                    for kc in range(KC):
                        nc.tensor.matmul(ps_q, wq_sl[:, kc, :],
                                         xT_sb[:, kc, 0:SQ],
                                         start=(kc == 0), stop=(kc == KC - 1))
                    nc.vector.tensor_scalar_add(qT_sb[:, dt, :], ps_q,
                                                bq_sb[:, dt:dt + 1])
                for ts in range(1, S // N512):
                    for kc in range(KC):
                        nc.sync.dma_start(
                            out=xT_sb[:, kc, ts * N512:(ts + 1) * N512],
                            in_=xT_v[:, kc, ts * N512:(ts + 1) * N512])

                prepare(0)
                v_open(0)
                attn(0, pre_t=lambda t: v_tt(0, t))
                prepare(1)
                v_open(1)
                for _tt in range(0, 4):
                    v_tt(1, _tt)
                attn(1)
                prepare(2)
                for _tt in range(4, 8):
                    v_tt(1, _tt)
                attn(2)
                prepare(3)
                for _tt in range(8, 12):
                    v_tt(1, _tt)
                attn(3)
                prepare(4)
                for _tt in range(12, NT):
                    v_tt(1, _tt)
                attn(4)
                prepare(5)
                attn(5)
                prepare(6)
                attn(6)
                prepare(7)
                attn(7)

            # ------------- epilogue: all in transposed [DM, 512] ----------
            with tc.tile_pool(name="ep", bufs=2) as ep, \
                 tc.tile_pool(name="epo", bufs=2) as epo, \
                 tc.tile_pool(name="lns", bufs=1) as lns, \
                 tc.tile_pool(name="spp", bufs=2, space="PSUM") as spp:
                # own x^T for the residual (own cols are first in xT)
                xTr = lns.tile([NP, KC, SQ], F32R)
                for kc in range(KC):
                    nc.sync.dma_start(out=xTr[:, kc, :],
                                      in_=xT_v[:, kc, 0:SQ])
                # gate = sigmoid(z)  (in place)
                for g in range(4):
                    nc.scalar.activation(out=gateT_acc[:, 2 * g:2 * g + 2, :],
                                         in_=gateT_acc[:, 2 * g:2 * g + 2, :],
                                         func=AF.Sigmoid)
                musum_ps = spp.tile([NP, SQ], F32, tag="ms", name="musum")
                sqsum_ps = spp.tile([NP, SQ], F32, tag="ms", name="sqsum")
                for dt in range(KC):
                    # ypre = gate*(out - x) + 2x  (overwrites outT_acc)
                    t1 = ep.tile([NP, SQ], F32, tag="t1", name="t1")
                    nc.vector.tensor_sub(t1, outT_acc[:, dt, :], xTr[:, dt, :])
                    nc.vector.tensor_mul(t1, t1, gateT_acc[:, dt, :])
                    nc.vector.scalar_tensor_tensor(
                        out=outT_acc[:, dt, :], in0=xTr[:, dt, :], scalar=2.0,
                        in1=t1, op0=ALU.mult, op1=ALU.add)
                    sq = ep.tile([NP, SQ], F32R, tag="sq", name="sq")
                    nc.vector.tensor_mul(sq, outT_acc[:, dt, :],
                                         outT_acc[:, dt, :])
                    nc.tensor.matmul(musum_ps, ones_sb, outT_acc[:, dt, :],
                                     start=(dt == 0), stop=(dt == KC - 1))
                    nc.tensor.matmul(sqsum_ps, ones_sb, sq,
                                     start=(dt == 0), stop=(dt == KC - 1))
                mu_sb = lns.tile([NP, SQ], F32)
                m2_sb = lns.tile([NP, SQ], F32)
                var_sb = lns.tile([NP, SQ], F32)
                rstd_sb = lns.tile([NP, SQ], F32)
                nc.vector.tensor_scalar_mul(mu_sb, musum_ps, 1.0 / DM)
                nc.vector.tensor_scalar_mul(m2_sb, sqsum_ps, 1.0 / DM)
                nc.vector.tensor_mul(var_sb, mu_sb, mu_sb)
                nc.vector.tensor_sub(var_sb, m2_sb, var_sb)
                nc.scalar.activation(out=var_sb, in_=var_sb, func=AF.Sqrt,
                                     bias=eps_sb)
                nc.vector.reciprocal(rstd_sb, var_sb)
                for dt in range(KC):
                    yt = epo.tile([NP, SQ], F32, tag="y", name="yt")
                    nc.vector.tensor_sub(yt, outT_acc[:, dt, :], mu_sb)
                    nc.vector.tensor_mul(yt, yt, rstd_sb)
                    nc.vector.tensor_scalar(
                        out=yt, in0=yt,
                        scalar1=gam_sb[:, dt:dt + 1],
                        scalar2=bet_sb[:, dt:dt + 1],
                        op0=ALU.mult, op1=ALU.add)
                    nc.sync.dma_start(out=y_v[:, dt, :], in_=yt)

    nc.compile()
    return nc


def kernel(x, Wq, bq, Wk, bk, Wv, bv, Wo, bo, Wg, bg, attention_weights,
           ln_gamma, ln_beta):
    x = np.asarray(x, dtype=np.float32)
    f32 = lambda a: np.ascontiguousarray(np.asarray(a, dtype=np.float32))
    Wq, Wk, Wv, Wo, Wg = map(f32, (Wq, Wk, Wv, Wo, Wg))
    bq, bk, bv, bo, bg = map(f32, (bq, bk, bv, bo, bg))
    aw, gam, bet = map(f32, (attention_weights, ln_gamma, ln_beta))

    if "nc" not in _CACHE:
        _CACHE["nc"] = _build()
    nc = _CACHE["nc"]

    # fold softmax(attention_weights) into Wv / bv
    e = np.exp(aw - aw.max())
    head_w = (e / e.sum()).astype(np.float32)
    hw_exp = np.repeat(head_w, HD)              # [DM]
    Wv_s = Wv * hw_exp[:, None]
    bv_s = bv * hw_exp

    # fold gate projection through the output projection
    Wgo = (Wg.astype(np.float64) @ Wo.astype(np.float64)).astype(np.float32)
    bgo = (Wg.astype(np.float64) @ bo.astype(np.float64)).astype(np.float32) + bg

    wqT = np.ascontiguousarray(Wq.T)
    wkT = np.ascontiguousarray(Wk.T)
    wvT = np.ascontiguousarray(Wv_s.T)
    woT = np.ascontiguousarray(Wo.T)
    wgoT = np.ascontiguousarray(Wgo.T)

    in_maps = []
    for c in range(8):
        b, blk = divmod(c, 4)
        r0 = blk * SQ
        xb = x[b]
        perm = np.r_[r0:r0 + SQ, 0:r0, r0 + SQ:S]
        in_maps.append({
            "xT": np.ascontiguousarray(xb[perm].T),
            "wkT": wkT, "wvT": wvT, "wqT": wqT, "woT": woT, "wgoT": wgoT,
            "bq": bq, "bk": bk, "bv": bv_s, "bo": bo, "bgo": bgo,
            "gam": gam, "bet": bet,
        })

    last_exc = None
    for _attempt in range(3):
        try:
            res = run_bass_kernel_spmd(nc, in_maps, core_ids=list(range(8)),
                                       trace=_TRACE[0])
            break
        except Exception as exc:  # flaky NRT_EXEC_UNIT errors: retry
            last_exc = exc
            import time
            time.sleep(2.0)
    else:
        raise last_exc
    _LAST_RESULT[0] = res

    y = np.empty((B, S, DM), dtype=np.float32)
    for c in range(8):
        b, blk = divmod(c, 4)
        r0 = blk * SQ
        y[b, r0:r0 + SQ] = res.results[c]["y"].T
    return y


# revision 18
# speedup vs baseline: 1.1684x; 1.1684x over previous
"""Trainium2 Bass kernel for EnhancedMultiHeadAttention (B=2, S=2048, DM=1024, H=16).

Sharding: 8 NeuronCores = 2 batches x 4 query-row blocks of 512 rows. Each
core computes K/V for its whole batch (4x redundant; cheaper than the
~30us-per-op AllGather firmware cost measured on this runtime), plus
attention, output projection, gate and layernorm for its own 512 query rows.
No collectives; the host concatenates the 8 output shards.

Schedule: one fused window. Q^T is projected up front; each head pair's K
rows are projected just-in-time; V column halves are projected into
SBUF-resident tiles, with pair 0's V tiles fused into its attention t-loop
so the ScalarE GELU stream (the ~148us serial bottleneck: 16.8M exact-erf
GELU elements/core at 1 elem/cycle/lane) starts as early as possible. All
projection matmuls fill PE slack underneath the GELU stream, which also
keeps the PE HAM clock un-throttled. The out-projection (both orientations:
row-major for the residual/LN epilogue, transposed as lhsT for the gate
matmul), gate, sigmoid, and the bn_stats-based layernorm trail the window,
pipelined per 128-row tile.

Precision: all matmuls run in fp32r (1 cycle/row at N>=256, ~1.5e-4 per
matmul) except attn @ v in bf16, which lets the two heads of a pair be
col-packed into one PSUM bank (fp32r cannot target PSUM partitions 64-127;
row-packed K=64 score matmuls are fine). The 1/sqrt(64) score scale rides
the GELU activation's free affine; softmax(attention_weights) is folded
into Wv/bv on the host; per-core query columns are permuted to the front of
x^T so Q projects from the same resident tile (t-order in attention is
permutation-invariant as long as K and V share it).

Measured on 8 axon-tunneled trn2 cores: HW exec ~382us, rel err 2.4e-4
(vs fp32 reference; fp64-reference check identical).
"""
import math
import os
import sys

import numpy as np

for _p in ("/opt/trn_rl_repo", "/opt/pypackages"):
    if _p not in sys.path:
        sys.path.append(_p)

import concourse.bass as bass
import concourse.mybir as mybir
import concourse.tile as tile
from concourse import bacc
from concourse.bass_utils import run_bass_kernel_spmd

F32R = mybir.dt.float32r
F32 = mybir.dt.float32
BF16 = mybir.dt.bfloat16
AF = mybir.ActivationFunctionType
ALU = mybir.AluOpType

B, S, DM, H = 2, 2048, 1024, 16
HD = DM // H                  # 64
SQ = 512                      # query rows per core
NP = 128                      # partitions
KC = DM // NP                 # 8 contraction chunks
NT = S // NP                  # 16 key/value tiles
NPAIR = H // 2                # 8 head pairs
NST = SQ // NP                # 4 row tiles in row-layout phases
N512 = 512
NQ = 512                      # v-projection column half width
SCALE = 1.0 / math.sqrt(HD)
EPS = 1e-5

_CACHE = {}
_TRACE = [False]
_LAST_RESULT = [None]


def _bcast(ap_1d, p=NP):
    return bass.AP(tensor=ap_1d.tensor, offset=ap_1d.offset,
                   ap=[[0, p]] + list(ap_1d.ap))


def _build():
    nc = bacc.Bacc("TRN2", target_bir_lowering=False, debug=False)

    xT_d = nc.dram_tensor("xT", [DM, S], F32R, kind="ExternalInput").ap()
    wkT_d = nc.dram_tensor("wkT", [DM, DM], F32R, kind="ExternalInput").ap()
    wvT_d = nc.dram_tensor("wvT", [DM, DM], F32R, kind="ExternalInput").ap()
    wqT_d = nc.dram_tensor("wqT", [DM, DM], F32R, kind="ExternalInput").ap()
    woT_d = nc.dram_tensor("woT", [DM, DM], F32R, kind="ExternalInput").ap()
    wgoT_d = nc.dram_tensor("wgoT", [DM, DM], F32R, kind="ExternalInput").ap()
    bq_d = nc.dram_tensor("bq", [DM], F32, kind="ExternalInput").ap()
    bk_d = nc.dram_tensor("bk", [DM], F32, kind="ExternalInput").ap()
    bv_d = nc.dram_tensor("bv", [DM], F32, kind="ExternalInput").ap()
    bo_d = nc.dram_tensor("bo", [DM], F32, kind="ExternalInput").ap()
    bgo_d = nc.dram_tensor("bgo", [DM], F32, kind="ExternalInput").ap()
    gam_d = nc.dram_tensor("gam", [DM], F32, kind="ExternalInput").ap()
    bet_d = nc.dram_tensor("bet", [DM], F32, kind="ExternalInput").ap()
    y_d = nc.dram_tensor("y", [DM, SQ], F32, kind="ExternalOutput").ap()

    xT_v = xT_d.rearrange("(c p) s -> p c s", p=NP)
    wk_v = wkT_d.rearrange("(c p) d -> p c d", p=NP)
    wv_v = wvT_d.rearrange("(c p) d -> p c d", p=NP)
    wq_v = wqT_d.rearrange("(c p) d -> p c d", p=NP)
    wo_v = woT_d.rearrange("(c p) d -> p c d", p=NP)
    wgo_v = wgoT_d.rearrange("(c p) d -> p c d", p=NP)
    y_v = y_d.rearrange("(c p) s -> p c s", p=NP)

    with tile.TileContext(nc) as tc:
        with tc.tile_pool(name="pers", bufs=1) as pers, \
             tc.tile_pool(name="acc", bufs=1) as acc:
            bq_sb = pers.tile([NP, KC], F32)
            bk_sb = pers.tile([NP, KC], F32)
            bo_sb = pers.tile([NP, KC], F32)
            bgo_sb = pers.tile([NP, KC], F32)
            gam_sb = pers.tile([NP, KC], F32)
            bet_sb = pers.tile([NP, KC], F32)
            nc.sync.dma_start(out=bq_sb, in_=bq_d.rearrange("(c p) -> p c", p=NP))
            nc.sync.dma_start(out=bk_sb, in_=bk_d.rearrange("(c p) -> p c", p=NP))
            nc.sync.dma_start(out=bo_sb, in_=bo_d.rearrange("(c p) -> p c", p=NP))
            nc.sync.dma_start(out=bgo_sb, in_=bgo_d.rearrange("(c p) -> p c", p=NP))
            nc.sync.dma_start(out=gam_sb, in_=gam_d.rearrange("(c p) -> p c", p=NP))
            nc.sync.dma_start(out=bet_sb, in_=bet_d.rearrange("(c p) -> p c", p=NP))
            bv_bc = pers.tile([NP, DM], F32)
            nc.sync.dma_start(out=bv_bc, in_=_bcast(bv_d))
            eps_sb = pers.tile([NP, 1], F32)
            nc.vector.memset(eps_sb, EPS)

            ones_f32 = pers.tile([NP, NP], F32)
            nc.vector.memset(ones_f32, 1.0)
            ones_sb = pers.tile([NP, NP], F32R)
            nc.gpsimd.dma_start(out=ones_sb, in_=ones_f32)
            outT_acc = acc.tile([NP, KC, SQ], F32R)
            gateT_acc = acc.tile([NP, KC, SQ], F32R)

            with tc.tile_pool(name="xres", bufs=1) as xres, \
                 tc.tile_pool(name="qres", bufs=1) as qres, \
                 tc.tile_pool(name="wvp", bufs=1) as wvp, \
                 tc.tile_pool(name="wsl", bufs=2) as wsl, \
                 tc.tile_pool(name="kpp", bufs=2) as kpp, \
                 tc.tile_pool(name="wop", bufs=2) as wop, \
                 tc.tile_pool(name="ctxp", bufs=2) as ctxp, \
                 tc.tile_pool(name="vqp", bufs=2) as vqp, \
                 tc.tile_pool(name="attp", bufs=3) as attp, \
                 tc.tile_pool(name="pp", bufs=2, space="PSUM") as pp, \
                 tc.tile_pool(name="scop", bufs=2, space="PSUM") as scop, \
                 tc.tile_pool(name="cxp", bufs=2, space="PSUM") as cxp:
                xT_sb = xres.tile([NP, KC, S], F32R)
                qT_sb = qres.tile([NP, KC, SQ], BF16)

                v_q = [None] * 2

                wv_sbs = [None] * 2

                def v_open(q):
                    wv_sb = wvp.tile([NP, KC, NQ], F32R, tag="wv", name="wv_sb")
                    nc.gpsimd.dma_start(out=wv_sb,
                                        in_=wv_v[:, :, q * NQ:(q + 1) * NQ])
                    wv_sbs[q] = wv_sb
                    vq = vqp.tile([NP, NT, NQ], BF16, tag="vq", name="vq")
                    v_q[q] = vq

                def v_tt(q, tt):
                    ps_t = pp.tile([NP, NQ], F32, tag="pj", name="ps_t")
                    for kc in range(KC):
                        nc.tensor.matmul(
                            ps_t,
                            xT_sb[:, kc, tt * NP:(tt + 1) * NP],
                            wv_sbs[q][:, kc, :],
                            start=(kc == 0), stop=(kc == KC - 1))
                    nc.vector.tensor_add(
                        v_q[q][:, tt, :], ps_t, bv_bc[:, q * NQ:(q + 1) * NQ])

                kpairs = [None] * NPAIR

                wk_tiles = {}

                def wk_dma(p):
                    wk_sl = wsl.tile([NP, KC, NP], F32R, tag="wk", name="wk_sl")
                    nc.sync.dma_start(out=wk_sl,
                                      in_=wk_v[:, :, p * NP:(p + 1) * NP])
                    wk_tiles[p] = wk_sl

                def prepare(p, inc_ops=None):
                    # kpair[d, t] = sum_k Wk[d, k] x[t, k] + bk[d], d in pair rows
                    if p not in wk_tiles:
                        wk_dma(p)
                    wk_sl = wk_tiles[p]
                    kpair = kpp.tile([NP, S], BF16, tag="kp", name="kpair")
                    for ts in range(S // N512):
                        ps_t = pp.tile([NP, N512], F32, tag="pj", name="ps_t")
                        for kc in range(KC):
                            nc.tensor.matmul(
                                ps_t,
                                wk_sl[:, kc, :],
                                xT_sb[:, kc, ts * N512:(ts + 1) * N512],
                                start=(kc == 0), stop=(kc == KC - 1))
                        nc.vector.tensor_scalar_add(
                            kpair[:, ts * N512:(ts + 1) * N512], ps_t,
                            bk_sb[:, p:p + 1])
                        if inc_ops:
                            for op in inc_ops[ts * 4:(ts + 1) * 4]:
                                op()
                    kpairs[p] = kpair

                def attn(p, pre_t=None):
                    kpair = kpairs[p]
                    vq = v_q[p // 4]
                    c0 = (p % 4) * NP
                    wo_sl = wop.tile([NP, DM], F32R, tag="wo", name="wo_sl")
                    nc.sync.dma_start(out=wo_sl, in_=wo_v[:, p, :])
                    wgo_sl = wop.tile([NP, DM], F32R, tag="wo", name="wgo_sl")
                    nc.sync.dma_start(out=wgo_sl, in_=wgo_v[:, p, :])
                    ctx_ps = cxp.tile([NP, SQ], F32, tag="cx", name="ctx_ps")
                    for t in range(NT):
                        if pre_t is not None:
                            pre_t(t)
                        sco = scop.tile([NP, 2 * SQ], F32, tag="sc", name="sco")
                        nc.tensor.matmul(sco[:, 0:SQ],
                                         kpair[0:64, t * NP:(t + 1) * NP],
                                         qT_sb[0:64, p, :],
                                         start=True, stop=True,
                                         tile_position=(0, 0))
                        nc.tensor.matmul(sco[:, SQ:2 * SQ],
                                         kpair[64:128, t * NP:(t + 1) * NP],
                                         qT_sb[64:128, p, :],
                                         start=True, stop=True,
                                         tile_position=(64, 0))
                        att_t = attp.tile([NP, 2 * SQ], BF16, tag="at", name="att_t")
                        nc.scalar.activation(out=att_t, in_=sco, func=AF.Gelu,
                                             scale=SCALE)
                        nc.tensor.matmul(ctx_ps[0:64, :], vq[:, t, c0:c0 + 64],
                                         att_t[:, 0:SQ],
                                         start=(t == 0), stop=(t == NT - 1),
                                         tile_position=(0, 0))
                        nc.tensor.matmul(ctx_ps[64:128, :], vq[:, t, c0 + 64:c0 + NP],
                                         att_t[:, SQ:2 * SQ],
                                         start=(t == 0), stop=(t == NT - 1),
                                         tile_position=(0, 64))
                    ctx_sb = ctxp.tile([NP, SQ], F32R, tag="cs", name="ctx_sb")
                    nc.vector.tensor_copy(ctx_sb, ctx_ps)

                    def inc_o(dt, p=p, wo_sl=wo_sl, ctx_sb=ctx_sb):
                        ps_o = pp.tile([NP, SQ], F32, tag="pj", name="ps_o")
                        nc.tensor.matmul(ps_o, wo_sl[:, dt * NP:(dt + 1) * NP],
                                         ctx_sb, start=True, stop=True)
                        if p == 0:
                            nc.vector.tensor_scalar_add(
                                outT_acc[:, dt, :], ps_o, bo_sb[:, dt:dt + 1])
                        else:
                            nc.vector.tensor_add(
                                outT_acc[:, dt, :], ps_o, outT_acc[:, dt, :])

                    def inc_g(dt, p=p, wgo_sl=wgo_sl, ctx_sb=ctx_sb):
                        ps_g = pp.tile([NP, SQ], F32, tag="pj", name="ps_g")
                        nc.tensor.matmul(ps_g, wgo_sl[:, dt * NP:(dt + 1) * NP],
                                         ctx_sb, start=True, stop=True)
                        if p == 0:
                            nc.vector.tensor_scalar_add(
                                gateT_acc[:, dt, :], ps_g, bgo_sb[:, dt:dt + 1])
                        else:
                            nc.vector.tensor_add(
                                gateT_acc[:, dt, :], ps_g, gateT_acc[:, dt, :])

                    ops = []
                    for dt in range(KC):
                        ops.append(lambda dt=dt: inc_o(dt))
                        ops.append(lambda dt=dt: inc_g(dt))
                    return ops

                # opening DMA order: wk(0), xT query-cols chunk, then the
                # Q projection (wq slices land right behind), then rest of xT
                wk_dma(0)
                for kc in range(KC):
                    nc.sync.dma_start(
                        out=xT_sb[:, kc, 0:N512], in_=xT_v[:, kc, 0:N512])
                for dt in range(KC):
                    wq_sl = wsl.tile([NP, KC, NP], F32R, tag="wq", name="wq_sl")
                    nc.sync.dma_start(out=wq_sl,
                                      in_=wq_v[:, :, dt * NP:(dt + 1) * NP])
                    ps_q = pp.tile([NP, SQ], F32, tag="pj", name="ps_q")
                    for kc in range(KC):
                        nc.tensor.matmul(ps_q, wq_sl[:, kc, :],
                                         xT_sb[:, kc, 0:SQ],
                                         start=(kc == 0), stop=(kc == KC - 1))
                    nc.vector.tensor_scalar_add(qT_sb[:, dt, :], ps_q,
                                                bq_sb[:, dt:dt + 1])
                for ts in range(1, S // N512):
                    for kc in range(KC):
                        nc.sync.dma_start(
                            out=xT_sb[:, kc, ts * N512:(ts + 1) * N512],
                            in_=xT_v[:, kc, ts * N512:(ts + 1) * N512])

                prepare(0)
                v_open(0)
                io = attn(0, pre_t=lambda t: v_tt(0, t))
                prepare(1, io)
                v_open(1)
                for _tt in range(0, 4):
                    v_tt(1, _tt)
                io = attn(1)
                prepare(2, io)
                for _tt in range(4, 8):
                    v_tt(1, _tt)
                io = attn(2)
                prepare(3, io)
                for _tt in range(8, 12):
                    v_tt(1, _tt)
                io = attn(3)
                prepare(4, io)
                for _tt in range(12, NT):
                    v_tt(1, _tt)
                io = attn(4)
                prepare(5, io)
                io = attn(5)
                prepare(6, io)
                io = attn(6)
                prepare(7, io)
                io = attn(7)
                for op in io:
                    op()

            # ------------- epilogue: all in transposed [DM, 512] ----------
            with tc.tile_pool(name="ep", bufs=2) as ep, \
                 tc.tile_pool(name="epo", bufs=2) as epo, \
                 tc.tile_pool(name="lns", bufs=1) as lns, \
                 tc.tile_pool(name="spp", bufs=2, space="PSUM") as spp:
                # own x^T for the residual (own cols are first in xT)
                xTr = lns.tile([NP, KC, SQ], F32R)
                for kc in range(KC):
                    nc.sync.dma_start(out=xTr[:, kc, :],
                                      in_=xT_v[:, kc, 0:SQ])
                for g in range(4):
                    nc.scalar.activation(out=gateT_acc[:, 2 * g:2 * g + 2, :],
                                         in_=gateT_acc[:, 2 * g:2 * g + 2, :],
                                         func=AF.Sigmoid)
                musum_ps = spp.tile([NP, SQ], F32, tag="ms", name="musum")
                sqsum_ps = spp.tile([NP, SQ], F32, tag="ms", name="sqsum")
                for dt in range(KC):
                    t1 = ep.tile([NP, SQ], F32, tag="t1", name="t1")
                    nc.vector.tensor_sub(t1, outT_acc[:, dt, :], xTr[:, dt, :])
                    nc.vector.tensor_mul(t1, t1, gateT_acc[:, dt, :])
                    nc.vector.scalar_tensor_tensor(
                        out=outT_acc[:, dt, :], in0=xTr[:, dt, :], scalar=2.0,
                        in1=t1, op0=ALU.mult, op1=ALU.add)
                    sq = ep.tile([NP, SQ], F32R, tag="sq", name="sq")
                    nc.vector.tensor_mul(sq, outT_acc[:, dt, :],
                                         outT_acc[:, dt, :])
                    nc.tensor.matmul(musum_ps, ones_sb, outT_acc[:, dt, :],
                                     start=(dt == 0), stop=(dt == KC - 1))
                    nc.tensor.matmul(sqsum_ps, ones_sb, sq,
                                     start=(dt == 0), stop=(dt == KC - 1))
                mu_sb = lns.tile([NP, SQ], F32)
                m2_sb = lns.tile([NP, SQ], F32)
                var_sb = lns.tile([NP, SQ], F32)
                rstd_sb = lns.tile([NP, SQ], F32)
                nc.vector.tensor_scalar_mul(mu_sb, musum_ps, 1.0 / DM)
                nc.vector.tensor_scalar_mul(m2_sb, sqsum_ps, 1.0 / DM)
                nc.vector.tensor_mul(var_sb, mu_sb, mu_sb)
                nc.vector.tensor_sub(var_sb, m2_sb, var_sb)
                nc.scalar.activation(out=var_sb, in_=var_sb, func=AF.Sqrt,
                                     bias=eps_sb)
                nc.vector.reciprocal(rstd_sb, var_sb)
                for dt in range(KC):
                    yt = epo.tile([NP, SQ], F32, tag="y", name="yt")
                    nc.vector.tensor_sub(yt, outT_acc[:, dt, :], mu_sb)
                    nc.vector.tensor_mul(yt, yt, rstd_sb)
                    nc.vector.tensor_scalar(
                        out=yt, in0=yt,
                        scalar1=gam_sb[:, dt:dt + 1],
                        scalar2=bet_sb[:, dt:dt + 1],
                        op0=ALU.mult, op1=ALU.add)
                    nc.sync.dma_start(out=y_v[:, dt, :], in_=yt)

    nc.compile()
    return nc


def kernel(x, Wq, bq, Wk, bk, Wv, bv, Wo, bo, Wg, bg, attention_weights,
           ln_gamma, ln_beta):
    x = np.asarray(x, dtype=np.float32)
    f32 = lambda a: np.ascontiguousarray(np.asarray(a, dtype=np.float32))
    Wq, Wk, Wv, Wo, Wg = map(f32, (Wq, Wk, Wv, Wo, Wg))
    bq, bk, bv, bo, bg = map(f32, (bq, bk, bv, bo, bg))
    aw, gam, bet = map(f32, (attention_weights, ln_gamma, ln_beta))

    if "nc" not in _CACHE:
        _CACHE["nc"] = _build()
    nc = _CACHE["nc"]

    # fold softmax(attention_weights) into Wv / bv
    e = np.exp(aw - aw.max())
    head_w = (e / e.sum()).astype(np.float32)
    hw_exp = np.repeat(head_w, HD)              # [DM]
    Wv_s = Wv * hw_exp[:, None]
    bv_s = bv * hw_exp

    Wgo = (Wg.astype(np.float64) @ Wo.astype(np.float64)).astype(np.float32)
    bgo = (Wg.astype(np.float64) @ bo.astype(np.float64)).astype(np.float32) + bg

    wqT = np.ascontiguousarray(Wq.T)
    wkT = np.ascontiguousarray(Wk.T)
    wvT = np.ascontiguousarray(Wv_s.T)
    woT = np.ascontiguousarray(Wo.T)
    wgoT = np.ascontiguousarray(Wgo.T)

    in_maps = []
    for c in range(8):
        b, blk = divmod(c, 4)
        r0 = blk * SQ
        xb = x[b]
        perm = np.r_[r0:r0 + SQ, 0:r0, r0 + SQ:S]
        in_maps.append({
            "xT": np.ascontiguousarray(xb[perm].T),
            "wkT": wkT, "wvT": wvT, "wqT": wqT, "woT": woT, "wgoT": wgoT,
            "bq": bq, "bk": bk, "bv": bv_s, "bo": bo, "bgo": bgo,
            "gam": gam, "bet": bet,
        })

    last_exc = None
    for _attempt in range(3):
        try:
            res = run_bass_kernel_spmd(nc, in_maps, core_ids=list(range(8)),
                                       trace=_TRACE[0])
            break
        except Exception as exc:  # flaky NRT_EXEC_UNIT errors: retry
            last_exc = exc
            import time
            time.sleep(2.0)
    else:
        raise last_exc
    _LAST_RESULT[0] = res

    y = np.empty((B, S, DM), dtype=np.float32)
    for c in range(8):
        b, blk = divmod(c, 4)
        r0 = blk * SQ
        y[b, r0:r0 + SQ] = res.results[c]["y"].T
    return y

